# revision 22
# baseline (speedup 1.0000x reference)
"""Trainium2 Bass kernel for nn_RecursiveLogit (Bellman-Ford / max-plus on
8192 independent 64-node DAGs).

Algorithm
---------
The reference runs n_steps=63 synchronous Bellman-Ford iterations:
    value[i] = max_k( util[edge i->tgt_k] + value[tgt_k] ),  dest (node 63) = 0
Each graph is a DAG with all edges strictly forward (tgt > src), so 63
synchronous iterations converge exactly to the longest-path fixed point,
which a single backward sweep (i = 62 .. 0) computes with O(E) work instead
of O(63*E).

Device mapping (per NeuronCore, 1024 graphs, pure data parallelism):
 - partition p holds 8 graphs, split into 2 independent STREAMS of 4 graphs
   so the Pool engine (gather) and the DVE (add+max) ping-pong between
   streams instead of serializing on one per-level dependency chain.
 - per-edge utilities are computed on the DVE from host-permuted features
   (edges sorted by (graph, src, tgt)); parallel-edge dedup is a
   tensor_tensor_scan (segmented max) + one masked penalty op, processed in
   level-chunks so the sweep starts as soon as the first chunk is ready.
 - the per-level gather value[tgt] is a gpsimd local_scatter: the fp32 value
   table is bit-exactly split into uint16 halves (bitcast view) and
   scattered into the level's edge slots using per-partition indices
   (-1 entries are skipped). Indices ship as int8 and are expanded to the
   interleaved int16 form on the otherwise-idle ACT engine.
 - one scalar_tensor_tensor (util - C0 + gathered value) and one
   tensor_reduce(max) finish each level+stream, writing V[:, i, :].

The kernel takes FULL inputs and returns the FULL (value, util) outputs,
sharding graphs 1024-per-core across 8 NeuronCores internally.
"""

import os
import sys

import numpy as np

sys.path.insert(0, "/opt/trn_rl_repo")

# ---- problem constants (hardcoded; the harness always runs this shape) ----
N = 64          # nodes per graph
DEG = 8         # out-edges per non-dest node
NL = N - 1      # 63 levels / non-dest nodes per graph
EPG = NL * DEG  # 504 edges per graph
NG = 8192       # graphs
E = NG * EPG    # total edges
NCORES = 8
GPC = NG // NCORES   # 1024 graphs per core
NSTR = 2             # independent streams per partition
NEG = -1e9
PEN = -1.0e5    # penalty for non-group-last duplicate edge slots
CHL = 7         # levels per pipeline chunk (63 = 9 * 7)

_PROG_CACHE = {}


# =====================================================================
# host-side preparation
# =====================================================================

def _check_structure(feats, dest_mask, edge_index, W, b, n_steps):
    if feats.shape != (E, 4) or edge_index.shape != (2, E):
        return False
    if dest_mask.shape != (NG * N,):
        return False
    if int(n_steps) != NL:
        return False
    src = edge_index[0]
    tgt = edge_index[1]
    g = src // N
    if not np.array_equal(g, np.repeat(np.arange(NG, dtype=src.dtype), EPG)):
        return False
    if not np.array_equal(tgt // N, g):
        return False
    src_l = src - g * N
    pat = np.tile(np.repeat(np.arange(NL, dtype=src.dtype), DEG), NG)
    if not np.array_equal(src_l, pat):
        return False
    tgt_l = tgt - g * N
    if not ((tgt_l > src_l).all() and (tgt_l < N).all()):
        return False
    exp_dest = (np.arange(NG * N) % N) == (N - 1)
    if not np.array_equal(np.asarray(dest_mask, bool), exp_dest):
        return False
    return True


def _reference_fallback(feats, dest_mask, edge_index, W, b, n_steps):
    """Faithful numpy port of the reference; only used if the inputs do not
    match the documented generator structure."""
    n_nodes = dest_mask.shape[0]
    util = feats.astype(np.float32) @ W.T.astype(np.float32) + b.astype(np.float32)
    src, tgt = edge_index[0], edge_index[1]
    value = np.where(dest_mask[:, None], 0.0, NEG).astype(np.float32)
    for _ in range(int(n_steps)):
        msg = value[tgt] + util
        agg = np.full((n_nodes, 1), -np.inf, np.float32)
        np.maximum.at(agg, src, msg)
        agg = np.maximum(agg, NEG)
        value = np.where(dest_mask[:, None], 0.0, agg).astype(np.float32)
    return value, util.astype(np.float32)


def _prepare_host_p(feats, tgt_l3, W, b, ncores, Pv, Sv, NLv, nstr):
    """Sort edges, build per-core device input arrays.

    tgt_l3: [NGv, NLv, DEG] local targets. Graph g maps to core c = g // gpc,
    partition p, slot s with g = c*gpc + p*Sv + s; stream = s // sstr,
    s' = s % sstr. Device free-dim edge order is (l, stream, s', k) with
    level l = 0..NLv-1 meaning source node i = NLv-1-l.
    """
    NGv = tgt_l3.shape[0]
    gpc = NGv // ncores
    sstr = Sv // nstr
    TRI = (NLv * (NLv + 1)) // 2

    T = tgt_l3
    ordk = np.argsort(T, axis=2, kind="stable")
    Ts = np.take_along_axis(T, ordk, 2)                      # sorted targets
    base = (np.arange(NGv, dtype=np.int64) * NLv * DEG)[:, None, None]
    eg = base + (np.arange(NLv, dtype=np.int64) * DEG)[None, :, None] + ordk
    same_next = np.zeros((NGv, NLv, DEG), bool)
    same_next[:, :, :-1] = Ts[:, :, :-1] == Ts[:, :, 1:]
    cont = np.zeros((NGv, NLv, DEG), np.int8)
    cont[:, :, 1:] = (Ts[:, :, 1:] == Ts[:, :, :-1]).astype(np.int8)
    grouplast = ~same_next

    # ---- scatter index table -------------------------------------------
    # A[g, pos] = k-rank of the group-last edge for (level l, target j),
    # pos = l*(l+1)/2 + (j - i - 1); -1 where no edge.
    i_arr = np.arange(NLv)[None, :, None]                    # src node
    l_arr = (NLv - 1) - i_arr                                # level index
    pos = (l_arr * (l_arr + 1)) // 2 + (Ts - i_arr - 1)      # [NGv, NLv, DEG]
    A = np.full((NGv, TRI), -1, np.int8)
    gl = grouplast
    gidx = np.broadcast_to(np.arange(NGv)[:, None, None], pos.shape)[gl]
    kidx = np.broadcast_to(np.arange(DEG)[None, None, :], pos.shape)[gl]
    A[gidx, pos[gl]] = kidx.astype(np.int8)

    # graphs -> (core, p, stream, s'); idx value = s'*8 + k, or -1
    A_r = A.reshape(ncores, Pv, nstr, sstr, TRI)
    val8 = np.where(
        A_r >= 0,
        A_r + (np.arange(sstr, dtype=np.int8) * DEG)[None, None, None, :, None],
        np.int8(-1),
    )
    # idx8[c, p, stream, pos, s']
    idx8 = np.ascontiguousarray(
        val8.transpose(0, 1, 2, 4, 3).reshape(ncores, Pv, nstr * TRI * sstr)
    )

    # ---- permuted features / masks in (l, stream, s', k) order ---------
    eg_l = eg[:, ::-1, :]                                    # level-major
    egc = eg_l.reshape(ncores, Pv, nstr, sstr, NLv, DEG)     # (c,p,str,s',l,k)
    EW = NLv * Sv * DEG
    perm = egc.transpose(0, 1, 4, 2, 3, 5).reshape(ncores, Pv, EW)
    F = feats.astype(np.float32)[perm]                       # (c,p,EW,4)
    feats_p = np.ascontiguousarray(
        F.transpose(0, 1, 3, 2).reshape(ncores, Pv, 4 * EW)
    )
    cont_l = cont[:, ::-1, :].reshape(ncores, Pv, nstr, sstr, NLv, DEG)
    cont_p = np.ascontiguousarray(
        cont_l.transpose(0, 1, 4, 2, 3, 5).reshape(ncores, Pv, EW)
    )

    # ---- scalars --------------------------------------------------------
    Wf = W.astype(np.float32).reshape(4)
    bf = np.float32(np.asarray(b, np.float32).reshape(-1)[0])
    util_host_max = float(np.abs(feats.astype(np.float32) @ Wf + bf).max())
    C0 = np.float32(util_host_max + 1.0)
    wb = np.zeros((ncores, Pv, 8), np.float32)
    wb[:, :, 0:4] = Wf[None, None, :]
    wb[:, :, 4] = bf
    wb[:, :, 5] = C0
    wb[:, :, 6] = -C0
    return feats_p, idx8, cont_p, wb, perm


def _prepare_host(feats, edge_index, W, b):
    tgt_l = (edge_index[1] - edge_index[0] // N * N).astype(np.int32)
    return _prepare_host_p(
        feats, tgt_l.reshape(NG, NL, DEG), W, b, NCORES, 128, 8, NL, NSTR
    )


# =====================================================================
# device program
# =====================================================================

def _chunks(NLv):
    out = []
    for c0 in range(0, NLv, CHL):
        out.append(list(range(c0, min(c0 + CHL, NLv))))
    return out


def _build_program(P=128, S=8, NLv=NL, reps=1, nstr=NSTR):
    """Build the Bass/Tile program. Parameterized so a small variant can be
    simulated; production uses (128, 8, 63, 1, 2). reps>1 wraps the whole
    computation in an on-device For_i loop for wall-clock device timing."""
    import contextlib

    import concourse.bass as bass
    import concourse.mybir as mybir
    from concourse.bacc import Bacc
    from concourse.tile import TileContext

    f32 = mybir.dt.float32
    i16 = mybir.dt.int16
    i8 = mybir.dt.int8
    u16 = mybir.dt.uint16
    Alu = mybir.AluOpType
    Ax = mybir.AxisListType
    AFT = mybir.ActivationFunctionType

    SSTR = S // nstr                     # graph slots per stream
    EW = NLv * S * DEG                   # edge slots per partition
    LW = S * DEG                         # edge slots per level (all streams)
    SW = SSTR * DEG                      # edge slots per level per stream
    TRI = (NLv * (NLv + 1)) // 2
    VWS = (NLv + 1) * SSTR               # value-table width per stream (f32)
    chunks = _chunks(NLv)

    def w8l(l):                          # idx8 width of level l (one stream)
        return (l + 1) * SSTR

    def off8(l):                         # idx8 offset of level l within stream
        return (l * (l + 1)) // 2 * SSTR

    nc = Bacc()
    d_feats = nc.declare_dram_parameter("feats_p", [P, 4 * EW], f32, isOutput=False)
    d_idx = nc.declare_dram_parameter("idx8", [P, nstr * TRI * SSTR], i8, isOutput=False)
    d_cont = nc.declare_dram_parameter("cont", [P, EW], i8, isOutput=False)
    d_wb = nc.declare_dram_parameter("wb", [P, 8], f32, isOutput=False)
    d_vo = nc.declare_dram_parameter("value_o", [P, nstr * VWS], f32, isOutput=True)
    d_uo = nc.declare_dram_parameter("util_o", [P, EW], f32, isOutput=True)

    with TileContext(nc) as tc:
        with (
            tc.tile_pool(name="main", bufs=1) as pool,
            tc.tile_pool(name="lvl", bufs=4) as lp,
            tc.tile_pool(name="ichunk", bufs=3) as icp,
        ):
            feats_t = pool.tile([P, 4 * EW], f32)
            idx8_t = pool.tile([P, nstr * TRI * SSTR], i8)
            cont_t = pool.tile([P, EW], i8)
            wb_t = pool.tile([P, 8], f32)
            Uraw = pool.tile([P, EW], f32)
            Uwk = pool.tile([P, EW], f32)
            Vs = [pool.tile([P, VWS], f32, name=f"V{t}") for t in range(nstr)]

            nc.sync.dma_start(out=wb_t[:], in_=d_wb[:])
            nc.sync.dma_start(out=idx8_t[:], in_=d_idx[:])

            def w(c):
                return wb_t[:, c : c + 1]

            loop_ctx = tc.For_i(0, reps, 1) if reps > 1 else contextlib.nullcontext()
            with loop_ctx:
                for t in range(nstr):
                    nc.gpsimd.memset(Vs[t][:, NLv * SSTR : VWS], 0.0)
                V16 = [Vs[t][:].bitcast(u16) for t in range(nstr)]

                for ci, chunk in enumerate(chunks):
                    l0, l1 = chunk[0], chunk[-1] + 1
                    e0, e1 = l0 * LW, l1 * LW          # edge-slot range
                    # --- stream in this chunk's features / masks ---------
                    fap_s = d_feats[:].rearrange("p (c e) -> p c e", c=4)[:, :, e0:e1]
                    fap_d = feats_t[:].rearrange("p (c e) -> p c e", c=4)[:, :, e0:e1]
                    nc.sync.dma_start(out=fap_d, in_=fap_s)
                    nc.sync.dma_start(out=cont_t[:, e0:e1], in_=d_cont[:, e0:e1])

                    # --- util for this chunk on DVE ----------------------
                    def fch(c):
                        return feats_t[:, c * EW + e0 : c * EW + e1]

                    nc.vector.tensor_scalar(
                        out=Uraw[:, e0:e1], in0=fch(0),
                        scalar1=w(0), scalar2=w(4), op0=Alu.mult, op1=Alu.add,
                    )
                    for c in range(1, 4):
                        nc.vector.scalar_tensor_tensor(
                            out=Uraw[:, e0:e1], in0=fch(c),
                            scalar=w(c), in1=Uraw[:, e0:e1],
                            op0=Alu.mult, op1=Alu.add,
                        )
                    nc.sync.dma_start(out=d_uo[:, e0:e1], in_=Uraw[:, e0:e1])

                    # shift on ACT, segmented-max scan, duplicate penalty
                    nc.scalar.activation(
                        out=Uwk[:, e0:e1], in_=Uraw[:, e0:e1],
                        func=AFT.Identity, bias=w(5), scale=1.0,
                    )
                    nc.vector.tensor_tensor_scan(
                        out=Uwk[:, e0:e1], data0=cont_t[:, e0:e1],
                        data1=Uwk[:, e0:e1],
                        initial=0.0, op0=Alu.mult, op1=Alu.max,
                    )
                    nc.vector.scalar_tensor_tensor(
                        out=Uwk[:, e0 : e1 - 1], in0=cont_t[:, e0 + 1 : e1],
                        scalar=float(PEN), in1=Uwk[:, e0 : e1 - 1],
                        op0=Alu.mult, op1=Alu.add,
                    )

                    # --- expand int8 indices to interleaved int16 on ACT -
                    w8c = off8(l1) - off8(l0)          # chunk idx8 width
                    itiles = []
                    for t in range(nstr):
                        it = icp.tile([P, 2 * w8c], i16, tag=f"ic{t}")
                        src8 = idx8_t[:, t * TRI * SSTR + off8(l0) :
                                      t * TRI * SSTR + off8(l1)]
                        ev = it[:].rearrange("p (n two) -> p n two", two=2)
                        nc.scalar.activation(
                            out=ev[:, :, 0:1], in_=src8,
                            func=AFT.Copy, scale=2.0,
                        )
                        nc.scalar.activation(
                            out=ev[:, :, 1:2], in_=src8,
                            func=AFT.Identity, bias=1.0, scale=2.0,
                        )
                        # Pool observes the ACT build via a tiny copy so the
                        # scatters below carry a single (DVE) wait — the ISA
                        # encoding has one sync-wait slot.
                        ptok = lp.tile([P, 2], i16, tag=f"ptok{t}")
                        nc.gpsimd.tensor_copy(out=ptok[:], in_=it[:, 0:2])
                        itiles.append(it)

                    # --- the sweep: per level, per stream ----------------
                    for l in chunk:
                        i = NLv - 1 - l
                        loc = 2 * (off8(l) - off8(l0))
                        for t in range(nstr):
                            msg = lp.tile([P, SW], f32, tag=f"msg{t}")
                            msg16 = msg[:].bitcast(u16)
                            nc.gpsimd.local_scatter(
                                out_ap=msg16,
                                data_ap=V16[t][:, (i + 1) * 2 * SSTR : 2 * VWS],
                                idxs_ap=itiles[t][:, loc : loc + 2 * w8l(l)],
                                channels=P,
                                num_elems=2 * SW,
                                num_idxs=2 * w8l(l),
                            )
                            m2 = lp.tile([P, SW], f32, tag=f"m2{t}")
                            u0 = l * LW + t * SW
                            nc.vector.scalar_tensor_tensor(
                                out=m2[:], in0=Uwk[:, u0 : u0 + SW],
                                scalar=w(6), in1=msg[:],
                                op0=Alu.add, op1=Alu.add,
                            )
                            nc.vector.tensor_reduce(
                                out=Vs[t][:, i * SSTR : (i + 1) * SSTR],
                                in_=m2[:].rearrange("p (s k) -> p s k", k=DEG),
                                axis=Ax.X, op=Alu.max,
                            )

                for t in range(nstr):
                    nc.sync.dma_start(
                        out=d_vo[:, t * VWS : (t + 1) * VWS], in_=Vs[t][:]
                    )

    nc.finalize()
    return nc


def _get_program(key=(128, 8, NL, 1)):
    if key not in _PROG_CACHE:
        _PROG_CACHE[key] = _build_program(*key)
    return _PROG_CACHE[key]


# =====================================================================
# entry point
# =====================================================================

def _run_device(feats, edge_index, W, b, trace=False):
    from concourse.bass_utils import run_bass_kernel_spmd

    feats_p, idx8, cont_p, wb, perm = _prepare_host(feats, edge_index, W, b)
    nc = _get_program()

    in_maps = [
        {
            "feats_p": feats_p[c],
            "idx8": idx8[c],
            "cont": cont_p[c],
            "wb": wb[c],
        }
        for c in range(NCORES)
    ]
    res = run_bass_kernel_spmd(nc, in_maps, list(range(NCORES)), trace=trace)
    return res, perm


def _pjrt_loop_time(nc, in_maps, iters):
    """Build the sharded PJRT executable for `nc`, pre-stage inputs on the
    devices, and return the best wall time (s) of one execution."""
    import time as _time

    import jax
    import numpy as _np
    from jax.sharding import Mesh, NamedSharding, PartitionSpec

    try:
        from jax.experimental.shard_map import shard_map
    except Exception:
        from jax.shard_map import shard_map  # newer jax

    import concourse.mybir as mybir
    from concourse import bass2jax as b2j

    b2j.install_neuronx_cc_hook()
    partition_name = nc.partition_id_tensor.name if nc.partition_id_tensor else None
    in_names, out_names, out_avals, zero_outs = [], [], [], []
    for alloc in nc.m.functions[0].allocations:
        if not isinstance(alloc, mybir.MemoryLocationSet):
            continue
        name = alloc.memorylocations[0].name
        if alloc.kind == "ExternalInput":
            if name != partition_name:
                in_names.append(name)
        elif alloc.kind == "ExternalOutput":
            out_names.append(name)
            out_avals.append(
                jax.core.ShapedArray(tuple(alloc.tensor_shape), mybir.dt.np(alloc.dtype))
            )
            zero_outs.append(
                _np.zeros(tuple(alloc.tensor_shape), mybir.dt.np(alloc.dtype))
            )
    n_params = len(in_names)
    all_names = list(in_names) + list(out_names)
    if partition_name is not None:
        all_names.append(partition_name)

    def _body(*args):
        operands = list(args)
        if partition_name is not None:
            operands.append(b2j.partition_id_tensor())
        return tuple(
            b2j._bass_exec_p.bind(
                *operands,
                out_avals=tuple(out_avals),
                in_names=tuple(all_names),
                out_names=tuple(out_names),
                lowering_input_output_aliases=(),
                sim_require_finite=True,
                sim_require_nnan=True,
                nc=nc,
            )
        )

    n_outs = len(out_names)
    donate = tuple(range(n_params, n_params + n_outs))
    devices = jax.devices()[:NCORES]
    mesh = Mesh(_np.asarray(devices), ("core",))
    spec = PartitionSpec("core")
    sharded = jax.jit(
        shard_map(
            _body, mesh=mesh,
            in_specs=(spec,) * (n_params + n_outs),
            out_specs=(spec,) * n_outs,
            check_rep=False,
        ),
        donate_argnums=donate,
        keep_unused=True,
    )
    sh = NamedSharding(mesh, spec)
    concat_in = [
        jax.device_put(
            _np.concatenate([in_maps[c][nm] for c in range(NCORES)], axis=0), sh
        )
        for nm in in_names
    ]
    concat_zeros = [
        _np.zeros((NCORES * z.shape[0], *z.shape[1:]), z.dtype) for z in zero_outs
    ]
    # warm (compile + first exec)
    jax.block_until_ready(sharded(*concat_in, *concat_zeros))
    best = float("inf")
    for _ in range(iters):
        zs = [jax.device_put(z, sh) for z in concat_zeros]
        jax.block_until_ready(zs)
        t0 = _time.perf_counter()
        jax.block_until_ready(sharded(*concat_in, *zs))
        best = min(best, _time.perf_counter() - t0)
    return best


def timed_run(np_inputs, lo=8, hi=264, iters=6, verbose=True):
    """Estimate single-iteration device time by wall-clocking two on-device
    looped variants (For_i with `lo` and `hi` trip counts) and taking the
    delta — per-call dispatch overhead cancels. Returns ns."""
    feats_p, idx8, cont_p, wb, _ = _prepare_host(
        np.asarray(np_inputs["feats"]),
        np.asarray(np_inputs["edge_index"]),
        np.asarray(np_inputs["W"]),
        np.asarray(np_inputs["b"]),
    )
    in_maps = [
        {"feats_p": feats_p[c], "idx8": idx8[c], "cont": cont_p[c], "wb": wb[c]}
        for c in range(NCORES)
    ]
    walls = {}
    for reps in (lo, hi):
        nc = _get_program((128, 8, NL, reps))
        walls[reps] = _pjrt_loop_time(nc, in_maps, iters)
        if verbose:
            print(f"  loop reps={reps}: best wall {walls[reps] * 1e3:.2f} ms")
    return int((walls[hi] - walls[lo]) / (hi - lo) * 1e9)


def kernel(feats, dest_mask, edge_index, W, b, n_steps):
    feats = np.asarray(feats)
    edge_index = np.asarray(edge_index)
    W = np.asarray(W)
    b = np.asarray(b)
    if not _check_structure(feats, dest_mask, edge_index, W, b, n_steps):
        return _reference_fallback(feats, dest_mask, edge_index, W, b, n_steps)

    res, perm = _run_device(feats, edge_index, W, b)
    results = res.results

    # ---- assemble outputs ----------------------------------------------
    SSTR = 8 // NSTR
    VWS = N * SSTR
    value = np.empty((NG, N), np.float32)
    util = np.empty((E,), np.float32)
    for c in range(NCORES):
        vo = results[c]["value_o"]                 # [128, nstr*VWS + pad]
        # stream blocks: [p, t, node, s'] -> graph (c, p, t, s'), node
        vr = vo[:, : NSTR * VWS].reshape(128, NSTR, N, SSTR).transpose(0, 1, 3, 2)
        value[c * GPC : (c + 1) * GPC] = vr.reshape(GPC, N)
        util[perm[c].reshape(-1)] = results[c]["util_o"].reshape(-1)
    return value.reshape(NG * N, 1), util.reshape(E, 1)


# revision 34
# speedup vs baseline: 1.4769x; 1.4769x over previous
"""Trainium2 Bass kernel for nn_RecursiveLogit (Bellman-Ford / max-plus on
8192 independent 64-node DAGs).

Algorithm
---------
The reference runs n_steps=63 synchronous Bellman-Ford iterations:
    value[i] = max_k( util[edge i->tgt_k] + value[tgt_k] ),  dest (node 63) = 0
Each graph is a DAG with all edges strictly forward (tgt > src), so 63
synchronous iterations converge exactly to the longest-path fixed point,
which a single backward sweep (i = 62 .. 0) computes with O(E) work instead
of O(63*E).

Device mapping (per NeuronCore, 1024 graphs, pure data parallelism):
 - partition p holds 8 graphs, split into 2 independent STREAMS of 4 graphs
   so the Pool engine (gather) and the DVE (add+max) ping-pong between
   streams instead of serializing on one per-level dependency chain.
 - per-edge utilities are computed on the DVE from host-permuted features
   (edges sorted by (graph, src, tgt)); parallel-edge dedup is a
   tensor_tensor_scan (segmented max) + one masked penalty op, processed in
   level-chunks so the sweep starts as soon as the first chunk is ready.
 - the per-level gather value[tgt] is a gpsimd local_scatter: the fp32 value
   table is bit-exactly split into uint16 halves (bitcast view) and
   scattered into the level's edge slots using per-partition indices
   (-1 entries are skipped). Indices ship as int8 and are expanded to the
   interleaved int16 form on the otherwise-idle ACT engine.
 - one scalar_tensor_tensor (util - C0 + gathered value) and one
   tensor_reduce(max) finish each level+stream, writing V[:, i, :].

The kernel takes FULL inputs and returns the FULL (value, util) outputs,
sharding graphs 1024-per-core across 8 NeuronCores internally.
"""

import os
import sys

import numpy as np

sys.path.insert(0, "/opt/trn_rl_repo")

# ---- problem constants (hardcoded; the harness always runs this shape) ----
N = 64          # nodes per graph
DEG = 8         # out-edges per non-dest node
NL = N - 1      # 63 levels / non-dest nodes per graph
EPG = NL * DEG  # 504 edges per graph
NG = 8192       # graphs
E = NG * EPG    # total edges
NCORES = 8
GPC = NG // NCORES   # 1024 graphs per core
NSTR = 2             # independent streams per partition
NEG = -1e9
PEN = -1.0e5    # penalty for non-group-last duplicate edge slots
CHL = 7         # levels per pipeline chunk (63 = 9 * 7)

_PROG_CACHE = {}


# =====================================================================
# host-side preparation
# =====================================================================

def _check_structure(feats, dest_mask, edge_index, W, b, n_steps):
    if feats.shape != (E, 4) or edge_index.shape != (2, E):
        return False
    if dest_mask.shape != (NG * N,):
        return False
    if int(n_steps) != NL:
        return False
    src = edge_index[0]
    tgt = edge_index[1]
    g = src // N
    if not np.array_equal(g, np.repeat(np.arange(NG, dtype=src.dtype), EPG)):
        return False
    if not np.array_equal(tgt // N, g):
        return False
    src_l = src - g * N
    pat = np.tile(np.repeat(np.arange(NL, dtype=src.dtype), DEG), NG)
    if not np.array_equal(src_l, pat):
        return False
    tgt_l = tgt - g * N
    if not ((tgt_l > src_l).all() and (tgt_l < N).all()):
        return False
    exp_dest = (np.arange(NG * N) % N) == (N - 1)
    if not np.array_equal(np.asarray(dest_mask, bool), exp_dest):
        return False
    return True


def _reference_fallback(feats, dest_mask, edge_index, W, b, n_steps):
    """Faithful numpy port of the reference; only used if the inputs do not
    match the documented generator structure."""
    n_nodes = dest_mask.shape[0]
    util = feats.astype(np.float32) @ W.T.astype(np.float32) + b.astype(np.float32)
    src, tgt = edge_index[0], edge_index[1]
    value = np.where(dest_mask[:, None], 0.0, NEG).astype(np.float32)
    for _ in range(int(n_steps)):
        msg = value[tgt] + util
        agg = np.full((n_nodes, 1), -np.inf, np.float32)
        np.maximum.at(agg, src, msg)
        agg = np.maximum(agg, NEG)
        value = np.where(dest_mask[:, None], 0.0, agg).astype(np.float32)
    return value, util.astype(np.float32)


def _prepare_host_p(feats, tgt_l3, W, b, ncores, Pv, Sv, NLv, nstr):
    """Sort edges, build per-core device input arrays.

    tgt_l3: [NGv, NLv, DEG] local targets. Graph g maps to core c = g // gpc,
    partition p, slot s with g = c*gpc + p*Sv + s; stream = s // sstr,
    s' = s % sstr. Device free-dim edge order is (l, stream, s', k) with
    level l = 0..NLv-1 meaning source node i = NLv-1-l.
    """
    NGv = tgt_l3.shape[0]
    gpc = NGv // ncores
    sstr = Sv // nstr
    TRI = (NLv * (NLv + 1)) // 2

    T = tgt_l3
    ordk = np.argsort(T, axis=2, kind="stable")
    Ts = np.take_along_axis(T, ordk, 2)                      # sorted targets
    base = (np.arange(NGv, dtype=np.int64) * NLv * DEG)[:, None, None]
    eg = base + (np.arange(NLv, dtype=np.int64) * DEG)[None, :, None] + ordk
    same_next = np.zeros((NGv, NLv, DEG), bool)
    same_next[:, :, :-1] = Ts[:, :, :-1] == Ts[:, :, 1:]
    cont = np.zeros((NGv, NLv, DEG), np.int8)
    cont[:, :, 1:] = (Ts[:, :, 1:] == Ts[:, :, :-1]).astype(np.int8)
    grouplast = ~same_next

    # ---- scatter index table -------------------------------------------
    # A[g, pos] = k-rank of the group-last edge for (level l, target j),
    # pos = l*(l+1)/2 + (j - i - 1); -1 where no edge.
    i_arr = np.arange(NLv)[None, :, None]                    # src node
    l_arr = (NLv - 1) - i_arr                                # level index
    pos = (l_arr * (l_arr + 1)) // 2 + (Ts - i_arr - 1)      # [NGv, NLv, DEG]
    A = np.full((NGv, TRI), -1, np.int8)
    gl = grouplast
    gidx = np.broadcast_to(np.arange(NGv)[:, None, None], pos.shape)[gl]
    kidx = np.broadcast_to(np.arange(DEG)[None, None, :], pos.shape)[gl]
    A[gidx, pos[gl]] = kidx.astype(np.int8)

    # graphs -> (core, p, stream, s'); idx value = s'*8 + k, or -1
    A_r = A.reshape(ncores, Pv, nstr, sstr, TRI)
    val8 = np.where(
        A_r >= 0,
        A_r + (np.arange(sstr, dtype=np.int8) * DEG)[None, None, None, :, None],
        np.int8(-1),
    )
    # idx8[c, p, stream, pos, s']
    idx8 = np.ascontiguousarray(
        val8.transpose(0, 1, 2, 4, 3).reshape(ncores, Pv, nstr * TRI * sstr)
    )

    # ---- permuted features / masks in (l, stream, s', k) order ---------
    eg_l = eg[:, ::-1, :]                                    # level-major
    egc = eg_l.reshape(ncores, Pv, nstr, sstr, NLv, DEG)     # (c,p,str,s',l,k)
    EW = NLv * Sv * DEG
    perm = egc.transpose(0, 1, 4, 2, 3, 5).reshape(ncores, Pv, EW)
    F = feats.astype(np.float32)[perm]                       # (c,p,EW,4)
    feats_p = np.ascontiguousarray(
        F.transpose(0, 1, 3, 2).reshape(ncores, Pv, 4 * EW)
    )
    cont_l = cont[:, ::-1, :].reshape(ncores, Pv, nstr, sstr, NLv, DEG)
    cont_p = np.ascontiguousarray(
        cont_l.transpose(0, 1, 4, 2, 3, 5).reshape(ncores, Pv, EW)
    )

    # ---- scalars --------------------------------------------------------
    Wf = W.astype(np.float32).reshape(4)
    bf = np.float32(np.asarray(b, np.float32).reshape(-1)[0])
    util_host_max = float(np.abs(feats.astype(np.float32) @ Wf + bf).max())
    C0 = np.float32(util_host_max + 1.0)
    wb = np.zeros((ncores, Pv, 8), np.float32)
    wb[:, :, 0:4] = Wf[None, None, :]
    wb[:, :, 4] = bf
    wb[:, :, 5] = C0
    wb[:, :, 6] = -C0
    return feats_p, idx8, cont_p, wb, perm


def _prepare_host(feats, edge_index, W, b):
    tgt_l = (edge_index[1] - edge_index[0] // N * N).astype(np.int32)
    return _prepare_host_p(
        feats, tgt_l.reshape(NG, NL, DEG), W, b, NCORES, 128, 8, NL, NSTR
    )


def _prepare_host_v4(feats, tgt_l3, W, b, ncores, Pv, Sv, NLv, nstr):
    """v4 layout: per-level table-shaped scatter DST (rows r = 63-tgt,
    cols s'), edge-side data = Uwk uint16 halves. Host ships:
      idx16 [P, nstr*NLv*2*SW]: per (stream, l, s', k, half): dst slot
            (r*SSTR + s')*2 + h for group-last edges, -1 otherwise
      tmask [P, nstr*TRI*SSTR]: per (stream, l, r, s'): 1 if no edge, else 0
    plus the v2 feats_p / cont_p / wb arrays (same edge order)."""
    NGv = tgt_l3.shape[0]
    sstr = Sv // nstr
    SW = sstr * DEG
    TRI = (NLv * (NLv + 1)) // 2
    Nn = NLv + 1

    feats_p, _idx8, cont_p, wb, perm = _prepare_host_p(
        feats, tgt_l3, W, b, ncores, Pv, Sv, NLv, nstr
    )

    T = tgt_l3
    ordk = np.argsort(T, axis=2, kind="stable")
    Ts = np.take_along_axis(T, ordk, 2)
    grouplast = np.ones((NGv, NLv, DEG), bool)
    grouplast[:, :, :-1] = Ts[:, :, :-1] != Ts[:, :, 1:]

    r = (Nn - 1) - Ts                                        # 63 - tgt
    # slot (within the level's table) = (r*sstr + s')*2 + h; s' added below
    slot8 = np.where(grouplast, r.astype(np.int32), -1)      # [NGv, NLv, DEG]

    # graphs -> (core, p, stream, s'); edge order (t, l, s', k); l = NLv-1-i
    s8 = slot8[:, ::-1, :].reshape(ncores, Pv, nstr, sstr, NLv, DEG)
    s8 = s8.transpose(0, 1, 2, 4, 3, 5)                      # (c,p,t,l,s',k)
    sp = np.arange(sstr, dtype=np.int32)[None, None, None, None, :, None]
    base = np.where(s8 >= 0, (s8 * sstr + sp) * 2, -10)
    idx16 = np.empty(base.shape + (2,), np.int16)
    idx16[..., 0] = base
    idx16[..., 1] = base + 1
    idx16[idx16 < 0] = -1
    idx16 = np.ascontiguousarray(
        idx16.reshape(ncores, Pv, nstr * NLv * 2 * SW)
    )

    # table mask: tmask[c, p, t, TRI(l)*sstr + r*sstr + s'] = 1 iff no edge
    gpc = NGv // ncores
    gall = np.arange(NGv)
    c_of = gall // gpc
    q = gall % gpc
    p_of = q // Sv
    s_of = q % Sv
    t_of = s_of // sstr
    sp_of = s_of % sstr
    i_arr = np.arange(NLv)[None, :, None]
    l_arr = (NLv - 1) - i_arr                                # [1, NLv, 1]
    posl = (l_arr * (l_arr + 1)) // 2
    pos = (posl + r) * sstr + sp_of[:, None, None]           # [NGv, NLv, DEG]
    tm = np.ones((ncores, Pv, nstr, TRI * sstr), np.int8)
    gl = grouplast
    gsel = np.broadcast_to(gall[:, None, None], pos.shape)[gl]
    tm[c_of[gsel], p_of[gsel], t_of[gsel], pos[gl]] = 0
    tmask = np.ascontiguousarray(tm.reshape(ncores, Pv, nstr * TRI * sstr))
    return feats_p, idx16, tmask, cont_p, wb, perm


# =====================================================================
# device program
# =====================================================================

def _chunks(NLv):
    out = []
    for c0 in range(0, NLv, CHL):
        out.append(list(range(c0, min(c0 + CHL, NLv))))
    return out


def _build_program(P=128, S=8, NLv=NL, reps=1, nstr=NSTR, mode="full"):
    """Build the Bass/Tile program. Parameterized so a small variant can be
    simulated; production uses (128, 8, 63, 1, 2). reps>1 wraps the whole
    computation in an on-device For_i loop for wall-clock device timing.
    mode: 'full' | 'noscat' (scatter->memset) | 'onedve' (skip per-level stt)
    — timing-bisect variants (wrong results)."""
    import contextlib

    import concourse.bass as bass
    import concourse.mybir as mybir
    from concourse.bacc import Bacc
    from concourse.tile import TileContext

    f32 = mybir.dt.float32
    i16 = mybir.dt.int16
    i8 = mybir.dt.int8
    u16 = mybir.dt.uint16
    Alu = mybir.AluOpType
    Ax = mybir.AxisListType
    AFT = mybir.ActivationFunctionType

    SSTR = S // nstr                     # graph slots per stream
    EW = NLv * S * DEG                   # edge slots per partition
    LW = S * DEG                         # edge slots per level (all streams)
    SW = SSTR * DEG                      # edge slots per level per stream
    TRI = (NLv * (NLv + 1)) // 2
    VWS = (NLv + 1) * SSTR               # value-table width per stream (f32)
    chunks = _chunks(NLv)

    def w8l(l):                          # idx8 width of level l (one stream)
        return (l + 1) * SSTR

    def off8(l):                         # idx8 offset of level l within stream
        return (l * (l + 1)) // 2 * SSTR

    nc = Bacc()
    d_feats = nc.declare_dram_parameter("feats_p", [P, 4 * EW], f32, isOutput=False)
    d_idx = nc.declare_dram_parameter("idx8", [P, nstr * TRI * SSTR], i8, isOutput=False)
    d_cont = nc.declare_dram_parameter("cont", [P, EW], i8, isOutput=False)
    d_wb = nc.declare_dram_parameter("wb", [P, 8], f32, isOutput=False)
    d_vo = nc.declare_dram_parameter("value_o", [P, nstr * VWS], f32, isOutput=True)
    d_uo = nc.declare_dram_parameter("util_o", [P, EW], f32, isOutput=True)

    with TileContext(nc) as tc:
        with (
            tc.tile_pool(name="main", bufs=1) as pool,
            tc.tile_pool(name="lvl", bufs=4) as lp,
            tc.tile_pool(name="ichunk", bufs=3) as icp,
        ):
            feats_t = pool.tile([P, 4 * EW], f32)
            idx8_t = pool.tile([P, nstr * TRI * SSTR], i8)
            cont_t = pool.tile([P, EW], i8)
            wb_t = pool.tile([P, 8], f32)
            Uraw = pool.tile([P, EW], f32)
            Uwk = pool.tile([P, EW], f32)
            Vs = [pool.tile([P, VWS], f32, name=f"V{t}") for t in range(nstr)]

            nc.sync.dma_start(out=wb_t[:], in_=d_wb[:])
            nc.sync.dma_start(out=idx8_t[:], in_=d_idx[:])

            def w(c):
                return wb_t[:, c : c + 1]

            loop_ctx = tc.For_i(0, reps, 1) if reps > 1 else contextlib.nullcontext()
            if mode == "empty":
                with loop_ctx:
                    nc.gpsimd.memset(Vs[0][:, 0:8], 0.0)
                for t in range(nstr):
                    nc.sync.dma_start(
                        out=d_vo[:, t * VWS : (t + 1) * VWS], in_=Vs[t][:]
                    )
                nc.sync.dma_start(out=d_uo[:, 0:EW], in_=Uwk[:])
                nc.finalize()
                return nc
            with loop_ctx:
                for t in range(nstr):
                    nc.gpsimd.memset(Vs[t][:, NLv * SSTR : VWS], 0.0)
                V16 = [Vs[t][:].bitcast(u16) for t in range(nstr)]

                for ci, chunk in enumerate(chunks):
                    l0, l1 = chunk[0], chunk[-1] + 1
                    e0, e1 = l0 * LW, l1 * LW          # edge-slot range
                    # --- stream in this chunk's features / masks ---------
                    fap_s = d_feats[:].rearrange("p (c e) -> p c e", c=4)[:, :, e0:e1]
                    fap_d = feats_t[:].rearrange("p (c e) -> p c e", c=4)[:, :, e0:e1]
                    nc.sync.dma_start(out=fap_d, in_=fap_s)
                    nc.sync.dma_start(out=cont_t[:, e0:e1], in_=d_cont[:, e0:e1])

                    # --- util for this chunk on DVE ----------------------
                    def fch(c):
                        return feats_t[:, c * EW + e0 : c * EW + e1]

                    nc.vector.tensor_scalar(
                        out=Uraw[:, e0:e1], in0=fch(0),
                        scalar1=w(0), scalar2=w(4), op0=Alu.mult, op1=Alu.add,
                    )
                    for c in range(1, 4):
                        nc.vector.scalar_tensor_tensor(
                            out=Uraw[:, e0:e1], in0=fch(c),
                            scalar=w(c), in1=Uraw[:, e0:e1],
                            op0=Alu.mult, op1=Alu.add,
                        )
                    nc.sync.dma_start(out=d_uo[:, e0:e1], in_=Uraw[:, e0:e1])

                    # shift on ACT, segmented-max scan, duplicate penalty
                    nc.scalar.activation(
                        out=Uwk[:, e0:e1], in_=Uraw[:, e0:e1],
                        func=AFT.Identity, bias=w(5), scale=1.0,
                    )
                    nc.vector.tensor_tensor_scan(
                        out=Uwk[:, e0:e1], data0=cont_t[:, e0:e1],
                        data1=Uwk[:, e0:e1],
                        initial=0.0, op0=Alu.mult, op1=Alu.max,
                    )
                    nc.vector.scalar_tensor_tensor(
                        out=Uwk[:, e0 : e1 - 1], in0=cont_t[:, e0 + 1 : e1],
                        scalar=float(PEN), in1=Uwk[:, e0 : e1 - 1],
                        op0=Alu.mult, op1=Alu.add,
                    )

                    # --- expand int8 indices to interleaved int16 on ACT -
                    w8c = off8(l1) - off8(l0)          # chunk idx8 width
                    itiles = []
                    for t in range(nstr):
                        it = icp.tile([P, 2 * w8c], i16, tag=f"ic{t}")
                        src8 = idx8_t[:, t * TRI * SSTR + off8(l0) :
                                      t * TRI * SSTR + off8(l1)]
                        ev = it[:].rearrange("p (n two) -> p n two", two=2)
                        nc.scalar.activation(
                            out=ev[:, :, 0:1], in_=src8,
                            func=AFT.Copy, scale=2.0,
                        )
                        nc.scalar.activation(
                            out=ev[:, :, 1:2], in_=src8,
                            func=AFT.Identity, bias=1.0, scale=2.0,
                        )
                        # Pool observes the ACT build via a tiny copy so the
                        # scatters below carry a single (DVE) wait — the ISA
                        # encoding has one sync-wait slot.
                        ptok = lp.tile([P, 2], i16, tag=f"ptok{t}")
                        nc.gpsimd.tensor_copy(out=ptok[:], in_=it[:, 0:2])
                        itiles.append(it)

                    # --- the sweep: per level, per stream ----------------
                    for l in chunk:
                        i = NLv - 1 - l
                        loc = 2 * (off8(l) - off8(l0))
                        for t in range(nstr):
                            msg = lp.tile([P, SW], f32, tag=f"msg{t}")
                            msg16 = msg[:].bitcast(u16)
                            if mode == "noscat":
                                nc.gpsimd.memset(msg16, 0)
                            else:
                                nc.gpsimd.local_scatter(
                                    out_ap=msg16,
                                    data_ap=V16[t][:, (i + 1) * 2 * SSTR : 2 * VWS],
                                    idxs_ap=itiles[t][:, loc : loc + 2 * w8l(l)],
                                    channels=P,
                                    num_elems=2 * SW,
                                    num_idxs=2 * w8l(l),
                                )
                            u0 = l * LW + t * SW
                            if mode == "onedve":
                                m2 = msg
                            else:
                                m2 = lp.tile([P, SW], f32, tag=f"m2{t}")
                                nc.vector.scalar_tensor_tensor(
                                    out=m2[:], in0=Uwk[:, u0 : u0 + SW],
                                    scalar=w(6), in1=msg[:],
                                    op0=Alu.add, op1=Alu.add,
                                )
                            nc.vector.tensor_reduce(
                                out=Vs[t][:, i * SSTR : (i + 1) * SSTR],
                                in_=m2[:].rearrange("p (s k) -> p s k", k=DEG),
                                axis=Ax.X, op=Alu.max,
                            )

                for t in range(nstr):
                    nc.sync.dma_start(
                        out=d_vo[:, t * VWS : (t + 1) * VWS], in_=Vs[t][:]
                    )

    nc.finalize()
    return nc


def _build_program_v4(P=128, S=8, NLv=NL, reps=1):
    """v4: per-level local_scatter writes edge UTILITIES (uint16 halves of
    util+C0) into a table-shaped dst indexed by (row r=63-tgt, graph slot);
    the DVE then does masked-penalty + V-add + max-reduce over the table.
    Scatters depend only on the util prep (not on V), so the Pool engine
    runs ahead freely and the per-level recurrence lives entirely inside
    the in-order DVE stream — no per-level cross-engine round trip."""
    import contextlib

    import concourse.bass as bass
    import concourse.mybir as mybir
    from concourse.bacc import Bacc
    from concourse.tile import TileContext

    f32 = mybir.dt.float32
    i16 = mybir.dt.int16
    i8 = mybir.dt.int8
    u16 = mybir.dt.uint16
    Alu = mybir.AluOpType
    Ax = mybir.AxisListType
    AFT = mybir.ActivationFunctionType

    EW = NLv * S * DEG                   # edge slots per partition
    LW = S * DEG                         # edge slots per level
    TRI = (NLv * (NLv + 1)) // 2
    VW = (NLv + 1) * S                   # value table (reversed rows)
    chunks = _chunks(NLv)

    nc = Bacc()
    d_feats = nc.declare_dram_parameter("feats_p", [P, 4 * EW], f32, isOutput=False)
    d_idx = nc.declare_dram_parameter("idx16", [P, NLv * 2 * LW], i16, isOutput=False)
    d_tm = nc.declare_dram_parameter("tmask", [P, TRI * S], i8, isOutput=False)
    d_cont = nc.declare_dram_parameter("cont", [P, EW], i8, isOutput=False)
    d_wb = nc.declare_dram_parameter("wb", [P, 8], f32, isOutput=False)
    d_vo = nc.declare_dram_parameter("value_o", [P, VW], f32, isOutput=True)
    d_uo = nc.declare_dram_parameter("util_o", [P, EW], f32, isOutput=True)

    with TileContext(nc) as tc:
        with (
            tc.tile_pool(name="main", bufs=1) as pool,
            tc.tile_pool(name="lvl", bufs=6) as lp,
        ):
            feats_t = pool.tile([P, 4 * EW], f32)
            idx_t = pool.tile([P, NLv * 2 * LW], i16)
            tm_t = pool.tile([P, TRI * S], i8)
            cont_t = pool.tile([P, EW], i8)
            wb_t = pool.tile([P, 8], f32)
            Uraw = pool.tile([P, EW], f32)
            Uwk = pool.tile([P, EW], f32)
            V = pool.tile([P, VW], f32)

            nc.sync.dma_start(out=wb_t[:], in_=d_wb[:])
            nc.sync.dma_start(out=idx_t[:], in_=d_idx[:])
            nc.sync.dma_start(out=tm_t[:], in_=d_tm[:])

            def w(c):
                return wb_t[:, c : c + 1]

            loop_ctx = tc.For_i(0, reps, 1) if reps > 1 else contextlib.nullcontext()
            with loop_ctx:
                nc.vector.memset(V[:, 0:S], 0.0)             # node 63 row
                Uwk16 = Uwk[:].bitcast(u16)

                for ci, chunk in enumerate(chunks):
                    l0, l1 = chunk[0], chunk[-1] + 1
                    e0, e1 = l0 * LW, l1 * LW
                    fap_s = d_feats[:].rearrange("p (c e) -> p c e", c=4)[:, :, e0:e1]
                    fap_d = feats_t[:].rearrange("p (c e) -> p c e", c=4)[:, :, e0:e1]
                    nc.sync.dma_start(out=fap_d, in_=fap_s)
                    nc.sync.dma_start(out=cont_t[:, e0:e1], in_=d_cont[:, e0:e1])

                    def fch(c):
                        return feats_t[:, c * EW + e0 : c * EW + e1]

                    nc.vector.tensor_scalar(
                        out=Uraw[:, e0:e1], in0=fch(0),
                        scalar1=w(0), scalar2=w(4), op0=Alu.mult, op1=Alu.add,
                    )
                    for c in range(1, 4):
                        nc.vector.scalar_tensor_tensor(
                            out=Uraw[:, e0:e1], in0=fch(c),
                            scalar=w(c), in1=Uraw[:, e0:e1],
                            op0=Alu.mult, op1=Alu.add,
                        )
                    nc.sync.dma_start(out=d_uo[:, e0:e1], in_=Uraw[:, e0:e1])
                    nc.scalar.activation(
                        out=Uwk[:, e0:e1], in_=Uraw[:, e0:e1],
                        func=AFT.Identity, bias=w(5), scale=1.0,
                    )
                    nc.vector.tensor_tensor_scan(
                        out=Uwk[:, e0:e1], data0=cont_t[:, e0:e1],
                        data1=Uwk[:, e0:e1],
                        initial=0.0, op0=Alu.mult, op1=Alu.max,
                    )
                    nc.vector.scalar_tensor_tensor(
                        out=Uwk[:, e0 : e1 - 1], in0=cont_t[:, e0 + 1 : e1],
                        scalar=float(PEN), in1=Uwk[:, e0 : e1 - 1],
                        op0=Alu.mult, op1=Alu.add,
                    )

                    for l in chunk:
                        wl = l + 1                       # table rows
                        posl = (l * (l + 1)) // 2
                        tbl = lp.tile([P, wl * S], f32, tag="tbl")
                        nc.gpsimd.local_scatter(
                            out_ap=tbl[:].bitcast(u16),
                            data_ap=Uwk16[:, l * LW * 2 : (l + 1) * LW * 2],
                            idxs_ap=idx_t[:, l * 2 * LW : (l + 1) * 2 * LW],
                            channels=P,
                            num_elems=2 * wl * S,
                            num_idxs=2 * LW,
                        )
                        nc.vector.scalar_tensor_tensor(
                            out=tbl[:], in0=tm_t[:, posl * S : (posl + wl) * S],
                            scalar=float(PEN), in1=tbl[:],
                            op0=Alu.mult, op1=Alu.add,
                        )
                        nc.vector.scalar_tensor_tensor(
                            out=tbl[:], in0=tbl[:],
                            scalar=w(6), in1=V[:, 0 : wl * S],
                            op0=Alu.add, op1=Alu.add,
                        )
                        nc.vector.tensor_reduce(
                            out=V[:, wl * S : (wl + 1) * S],
                            in_=tbl[:].rearrange("p (r s) -> p s r", s=S),
                            axis=Ax.X, op=Alu.max,
                        )

                nc.sync.dma_start(out=d_vo[:], in_=V[:])

    nc.finalize()
    return nc


def _get_program(key=(128, 8, NL, 1)):
    if key not in _PROG_CACHE:
        if len(key) > 4 and key[4] == "v4":
            _PROG_CACHE[key] = _build_program_v4(*key[:4])
        else:
            _PROG_CACHE[key] = _build_program(*key)
    return _PROG_CACHE[key]


# =====================================================================
# entry point
# =====================================================================

def _host_v4(feats, edge_index, W, b):
    tgt_l = (edge_index[1] - edge_index[0] // N * N).astype(np.int32)
    feats_p, idx16, tmask, cont_p, wb, perm = _prepare_host_v4(
        feats, tgt_l.reshape(NG, NL, DEG), W, b, NCORES, 128, 8, NL, 1
    )
    in_maps = [
        {
            "feats_p": feats_p[c],
            "idx16": idx16[c],
            "tmask": tmask[c],
            "cont": cont_p[c],
            "wb": wb[c],
        }
        for c in range(NCORES)
    ]
    return in_maps, perm


def _run_device(feats, edge_index, W, b, trace=False):
    from concourse.bass_utils import run_bass_kernel_spmd

    in_maps, perm = _host_v4(feats, edge_index, W, b)
    nc = _get_program((128, 8, NL, 1, "v4"))
    res = run_bass_kernel_spmd(nc, in_maps, list(range(NCORES)), trace=trace)
    return res, perm


def _pjrt_loop_time(nc, in_maps, iters):
    """Build the sharded PJRT executable for `nc`, pre-stage inputs on the
    devices, and return the best wall time (s) of one execution."""
    import time as _time

    import jax
    import numpy as _np
    from jax.sharding import Mesh, NamedSharding, PartitionSpec

    try:
        from jax.experimental.shard_map import shard_map
    except Exception:
        from jax.shard_map import shard_map  # newer jax

    import concourse.mybir as mybir
    from concourse import bass2jax as b2j

    b2j.install_neuronx_cc_hook()
    partition_name = nc.partition_id_tensor.name if nc.partition_id_tensor else None
    in_names, out_names, out_avals, zero_outs = [], [], [], []
    for alloc in nc.m.functions[0].allocations:
        if not isinstance(alloc, mybir.MemoryLocationSet):
            continue
        name = alloc.memorylocations[0].name
        if alloc.kind == "ExternalInput":
            if name != partition_name:
                in_names.append(name)
        elif alloc.kind == "ExternalOutput":
            out_names.append(name)
            out_avals.append(
                jax.core.ShapedArray(tuple(alloc.tensor_shape), mybir.dt.np(alloc.dtype))
            )
            zero_outs.append(
                _np.zeros(tuple(alloc.tensor_shape), mybir.dt.np(alloc.dtype))
            )
    n_params = len(in_names)
    all_names = list(in_names) + list(out_names)
    if partition_name is not None:
        all_names.append(partition_name)

    def _body(*args):
        operands = list(args)
        if partition_name is not None:
            operands.append(b2j.partition_id_tensor())
        return tuple(
            b2j._bass_exec_p.bind(
                *operands,
                out_avals=tuple(out_avals),
                in_names=tuple(all_names),
                out_names=tuple(out_names),
                lowering_input_output_aliases=(),
                sim_require_finite=True,
                sim_require_nnan=True,
                nc=nc,
            )
        )

    n_outs = len(out_names)
    donate = tuple(range(n_params, n_params + n_outs))
    devices = jax.devices()[:NCORES]
    mesh = Mesh(_np.asarray(devices), ("core",))
    spec = PartitionSpec("core")
    sharded = jax.jit(
        shard_map(
            _body, mesh=mesh,
            in_specs=(spec,) * (n_params + n_outs),
            out_specs=(spec,) * n_outs,
            check_rep=False,
        ),
        donate_argnums=donate,
        keep_unused=True,
    )
    sh = NamedSharding(mesh, spec)
    concat_in = [
        jax.device_put(
            _np.concatenate([in_maps[c][nm] for c in range(NCORES)], axis=0), sh
        )
        for nm in in_names
    ]
    concat_zeros = [
        _np.zeros((NCORES * z.shape[0], *z.shape[1:]), z.dtype) for z in zero_outs
    ]
    # warm (compile + first exec)
    jax.block_until_ready(sharded(*concat_in, *concat_zeros))
    best = float("inf")
    for _ in range(iters):
        zs = [jax.device_put(z, sh) for z in concat_zeros]
        jax.block_until_ready(zs)
        t0 = _time.perf_counter()
        jax.block_until_ready(sharded(*concat_in, *zs))
        best = min(best, _time.perf_counter() - t0)
    return best


def timed_run(np_inputs, lo=8, hi=264, iters=6, verbose=True, mode="full"):
    """Estimate single-iteration device time by wall-clocking two on-device
    looped variants (For_i with `lo` and `hi` trip counts) and taking the
    delta — per-call dispatch overhead cancels. Returns ns."""
    in_maps, _ = _host_v4(
        np.asarray(np_inputs["feats"]),
        np.asarray(np_inputs["edge_index"]),
        np.asarray(np_inputs["W"]),
        np.asarray(np_inputs["b"]),
    )
    walls = {}
    for reps in (lo, hi):
        nc = _get_program((128, 8, NL, reps, "v4"))
        walls[reps] = _pjrt_loop_time(nc, in_maps, iters)
        if verbose:
            print(f"  loop reps={reps}: best wall {walls[reps] * 1e3:.2f} ms")
    return int((walls[hi] - walls[lo]) / (hi - lo) * 1e9)


def kernel(feats, dest_mask, edge_index, W, b, n_steps):
    feats = np.asarray(feats)
    edge_index = np.asarray(edge_index)
    W = np.asarray(W)
    b = np.asarray(b)
    if not _check_structure(feats, dest_mask, edge_index, W, b, n_steps):
        return _reference_fallback(feats, dest_mask, edge_index, W, b, n_steps)

    res, perm = _run_device(feats, edge_index, W, b)
    results = res.results

    # ---- assemble outputs ----------------------------------------------
    value = np.empty((NG, N), np.float32)
    util = np.empty((E,), np.float32)
    for c in range(NCORES):
        vo = results[c]["value_o"]                 # [128, 64*8] reversed rows
        vr = vo.reshape(128, N, 8)[:, ::-1, :].transpose(0, 2, 1)
        value[c * GPC : (c + 1) * GPC] = vr.reshape(GPC, N)
        util[perm[c].reshape(-1)] = results[c]["util_o"].reshape(-1)
    return value.reshape(NG * N, 1), util.reshape(E, 1)


# revision 40
# speedup vs baseline: 1.7764x; 1.2028x over previous
"""Trainium2 Bass kernel for nn_RecursiveLogit (Bellman-Ford / max-plus on
8192 independent 64-node DAGs).

Algorithm
---------
The reference runs n_steps=63 synchronous Bellman-Ford iterations:
    value[i] = max_k( util[edge i->tgt_k] + value[tgt_k] ),  dest (node 63) = 0
Each graph is a DAG with all edges strictly forward (tgt > src), so 63
synchronous iterations converge exactly to the longest-path fixed point,
which a single backward sweep (i = 62 .. 0) computes with O(E) work instead
of O(63*E).

Device mapping (per NeuronCore, 1024 graphs, pure data parallelism):
 - partition p holds 8 graphs, split into 2 independent STREAMS of 4 graphs
   so the Pool engine (gather) and the DVE (add+max) ping-pong between
   streams instead of serializing on one per-level dependency chain.
 - per-edge utilities are computed on the DVE from host-permuted features
   (edges sorted by (graph, src, tgt)); parallel-edge dedup is a
   tensor_tensor_scan (segmented max) + one masked penalty op, processed in
   level-chunks so the sweep starts as soon as the first chunk is ready.
 - the per-level gather value[tgt] is a gpsimd local_scatter: the fp32 value
   table is bit-exactly split into uint16 halves (bitcast view) and
   scattered into the level's edge slots using per-partition indices
   (-1 entries are skipped). Indices ship as int8 and are expanded to the
   interleaved int16 form on the otherwise-idle ACT engine.
 - one scalar_tensor_tensor (util - C0 + gathered value) and one
   tensor_reduce(max) finish each level+stream, writing V[:, i, :].

The kernel takes FULL inputs and returns the FULL (value, util) outputs,
sharding graphs 1024-per-core across 8 NeuronCores internally.
"""

import os
import sys

import numpy as np

sys.path.insert(0, "/opt/trn_rl_repo")

# ---- problem constants (hardcoded; the harness always runs this shape) ----
N = 64          # nodes per graph
DEG = 8         # out-edges per non-dest node
NL = N - 1      # 63 levels / non-dest nodes per graph
EPG = NL * DEG  # 504 edges per graph
NG = 8192       # graphs
E = NG * EPG    # total edges
NCORES = 8
GPC = NG // NCORES   # 1024 graphs per core
NSTR = 2             # independent streams per partition
NEG = -1e9
PEN = -1.0e5    # penalty for non-group-last duplicate edge slots
CHL = 7         # levels per pipeline chunk (63 = 9 * 7)

_PROG_CACHE = {}


# =====================================================================
# host-side preparation
# =====================================================================

def _check_structure(feats, dest_mask, edge_index, W, b, n_steps):
    if feats.shape != (E, 4) or edge_index.shape != (2, E):
        return False
    if dest_mask.shape != (NG * N,):
        return False
    if int(n_steps) != NL:
        return False
    src = edge_index[0]
    tgt = edge_index[1]
    g = src // N
    if not np.array_equal(g, np.repeat(np.arange(NG, dtype=src.dtype), EPG)):
        return False
    if not np.array_equal(tgt // N, g):
        return False
    src_l = src - g * N
    pat = np.tile(np.repeat(np.arange(NL, dtype=src.dtype), DEG), NG)
    if not np.array_equal(src_l, pat):
        return False
    tgt_l = tgt - g * N
    if not ((tgt_l > src_l).all() and (tgt_l < N).all()):
        return False
    exp_dest = (np.arange(NG * N) % N) == (N - 1)
    if not np.array_equal(np.asarray(dest_mask, bool), exp_dest):
        return False
    return True


def _reference_fallback(feats, dest_mask, edge_index, W, b, n_steps):
    """Faithful numpy port of the reference; only used if the inputs do not
    match the documented generator structure."""
    n_nodes = dest_mask.shape[0]
    util = feats.astype(np.float32) @ W.T.astype(np.float32) + b.astype(np.float32)
    src, tgt = edge_index[0], edge_index[1]
    value = np.where(dest_mask[:, None], 0.0, NEG).astype(np.float32)
    for _ in range(int(n_steps)):
        msg = value[tgt] + util
        agg = np.full((n_nodes, 1), -np.inf, np.float32)
        np.maximum.at(agg, src, msg)
        agg = np.maximum(agg, NEG)
        value = np.where(dest_mask[:, None], 0.0, agg).astype(np.float32)
    return value, util.astype(np.float32)


def _prepare_host_p(feats, tgt_l3, W, b, ncores, Pv, Sv, NLv, nstr):
    """Sort edges, build per-core device input arrays.

    tgt_l3: [NGv, NLv, DEG] local targets. Graph g maps to core c = g // gpc,
    partition p, slot s with g = c*gpc + p*Sv + s; stream = s // sstr,
    s' = s % sstr. Device free-dim edge order is (l, stream, s', k) with
    level l = 0..NLv-1 meaning source node i = NLv-1-l.
    """
    NGv = tgt_l3.shape[0]
    gpc = NGv // ncores
    sstr = Sv // nstr
    TRI = (NLv * (NLv + 1)) // 2

    T = tgt_l3
    ordk = np.argsort(T, axis=2, kind="stable")
    Ts = np.take_along_axis(T, ordk, 2)                      # sorted targets
    base = (np.arange(NGv, dtype=np.int64) * NLv * DEG)[:, None, None]
    eg = base + (np.arange(NLv, dtype=np.int64) * DEG)[None, :, None] + ordk
    same_next = np.zeros((NGv, NLv, DEG), bool)
    same_next[:, :, :-1] = Ts[:, :, :-1] == Ts[:, :, 1:]
    cont = np.zeros((NGv, NLv, DEG), np.int8)
    cont[:, :, 1:] = (Ts[:, :, 1:] == Ts[:, :, :-1]).astype(np.int8)
    grouplast = ~same_next

    # ---- scatter index table -------------------------------------------
    # A[g, pos] = k-rank of the group-last edge for (level l, target j),
    # pos = l*(l+1)/2 + (j - i - 1); -1 where no edge.
    i_arr = np.arange(NLv)[None, :, None]                    # src node
    l_arr = (NLv - 1) - i_arr                                # level index
    pos = (l_arr * (l_arr + 1)) // 2 + (Ts - i_arr - 1)      # [NGv, NLv, DEG]
    A = np.full((NGv, TRI), -1, np.int8)
    gl = grouplast
    gidx = np.broadcast_to(np.arange(NGv)[:, None, None], pos.shape)[gl]
    kidx = np.broadcast_to(np.arange(DEG)[None, None, :], pos.shape)[gl]
    A[gidx, pos[gl]] = kidx.astype(np.int8)

    # graphs -> (core, p, stream, s'); idx value = s'*8 + k, or -1
    A_r = A.reshape(ncores, Pv, nstr, sstr, TRI)
    val8 = np.where(
        A_r >= 0,
        A_r + (np.arange(sstr, dtype=np.int8) * DEG)[None, None, None, :, None],
        np.int8(-1),
    )
    # idx8[c, p, stream, pos, s']
    idx8 = np.ascontiguousarray(
        val8.transpose(0, 1, 2, 4, 3).reshape(ncores, Pv, nstr * TRI * sstr)
    )

    # ---- permuted features / masks in (l, stream, s', k) order ---------
    eg_l = eg[:, ::-1, :]                                    # level-major
    egc = eg_l.reshape(ncores, Pv, nstr, sstr, NLv, DEG)     # (c,p,str,s',l,k)
    EW = NLv * Sv * DEG
    perm = egc.transpose(0, 1, 4, 2, 3, 5).reshape(ncores, Pv, EW)
    F = feats.astype(np.float32)[perm]                       # (c,p,EW,4)
    feats_p = np.ascontiguousarray(
        F.transpose(0, 1, 3, 2).reshape(ncores, Pv, 4 * EW)
    )
    cont_l = cont[:, ::-1, :].reshape(ncores, Pv, nstr, sstr, NLv, DEG)
    cont_p = np.ascontiguousarray(
        cont_l.transpose(0, 1, 4, 2, 3, 5).reshape(ncores, Pv, EW)
    )

    # ---- scalars --------------------------------------------------------
    Wf = W.astype(np.float32).reshape(4)
    bf = np.float32(np.asarray(b, np.float32).reshape(-1)[0])
    util_host_max = float(np.abs(feats.astype(np.float32) @ Wf + bf).max())
    C0 = np.float32(util_host_max + 1.0)
    wb = np.zeros((ncores, Pv, 8), np.float32)
    wb[:, :, 0:4] = Wf[None, None, :]
    wb[:, :, 4] = bf
    wb[:, :, 5] = C0
    wb[:, :, 6] = -C0
    return feats_p, idx8, cont_p, wb, perm


def _prepare_host(feats, edge_index, W, b):
    tgt_l = (edge_index[1] - edge_index[0] // N * N).astype(np.int32)
    return _prepare_host_p(
        feats, tgt_l.reshape(NG, NL, DEG), W, b, NCORES, 128, 8, NL, NSTR
    )


def _prepare_host_v4(feats, tgt_l3, W, b, ncores, Pv, Sv, NLv, nstr):
    """v4 layout: per-level table-shaped scatter DST (rows r = 63-tgt,
    cols s'), edge-side data = Uwk uint16 halves. Host ships:
      idx16 [P, nstr*NLv*2*SW]: per (stream, l, s', k, half): dst slot
            (r*SSTR + s')*2 + h for group-last edges, -1 otherwise
      tmask [P, nstr*TRI*SSTR]: per (stream, l, r, s'): 1 if no edge, else 0
    plus the v2 feats_p / cont_p / wb arrays (same edge order)."""
    NGv = tgt_l3.shape[0]
    sstr = Sv // nstr
    SW = sstr * DEG
    TRI = (NLv * (NLv + 1)) // 2
    Nn = NLv + 1

    feats_p, _idx8, cont_p, wb, perm = _prepare_host_p(
        feats, tgt_l3, W, b, ncores, Pv, Sv, NLv, nstr
    )

    T = tgt_l3
    ordk = np.argsort(T, axis=2, kind="stable")
    Ts = np.take_along_axis(T, ordk, 2)
    grouplast = np.ones((NGv, NLv, DEG), bool)
    grouplast[:, :, :-1] = Ts[:, :, :-1] != Ts[:, :, 1:]

    r = (Nn - 1) - Ts                                        # 63 - tgt
    # slot (within the level's table) = (r*sstr + s')*2 + h; s' added below
    slot8 = np.where(grouplast, r.astype(np.int32), -1)      # [NGv, NLv, DEG]

    # graphs -> (core, p, stream, s'); edge order (t, l, s', k); l = NLv-1-i
    s8 = slot8[:, ::-1, :].reshape(ncores, Pv, nstr, sstr, NLv, DEG)
    s8 = s8.transpose(0, 1, 2, 4, 3, 5)                      # (c,p,t,l,s',k)
    sp = np.arange(sstr, dtype=np.int32)[None, None, None, None, :, None]
    base = np.where(s8 >= 0, (s8 * sstr + sp) * 2, -10)
    idx16 = np.empty(base.shape + (2,), np.int16)
    idx16[..., 0] = base
    idx16[..., 1] = base + 1
    idx16[idx16 < 0] = -1
    idx16 = np.ascontiguousarray(
        idx16.reshape(ncores, Pv, nstr * NLv * 2 * SW)
    )

    # table mask over PADDED chunk tables: each level-l table is padded to
    # its chunk's max width Wc = l1*sstr rows; mask = 1 iff slot has no edge
    # (incl. pad rows). Flat layout: concat over chunks of [7, Wc*sstr].
    gpc = NGv // ncores
    gall = np.arange(NGv)
    c_of = gall // gpc
    q = gall % gpc
    p_of = q // Sv
    s_of = q % Sv
    t_of = s_of // sstr
    sp_of = s_of % sstr
    chunks = _chunks(NLv)
    # per-level offset into the padded-table flat array
    lvl_off = np.zeros(NLv, np.int64)
    off = 0
    wc_of = np.zeros(NLv, np.int64)
    for chunk in chunks:
        Wc = (chunk[-1] + 1) * sstr
        for l in chunk:
            lvl_off[l] = off
            wc_of[l] = Wc
            off += Wc
    TMW = int(off)
    i_arr = np.arange(NLv)[None, :, None]
    l_arr = (NLv - 1) - i_arr                                # [1, NLv, 1]
    pos = lvl_off[l_arr] + r * sstr + sp_of[:, None, None]   # [NGv, NLv, DEG]
    tm = np.ones((ncores, Pv, nstr, TMW), np.int8)
    gl = grouplast
    gsel = np.broadcast_to(gall[:, None, None], pos.shape)[gl]
    tm[c_of[gsel], p_of[gsel], t_of[gsel], pos[gl]] = 0
    tmask = np.ascontiguousarray(tm.reshape(ncores, Pv, nstr * TMW))
    return feats_p, idx16, tmask, cont_p, wb, perm


# =====================================================================
# device program
# =====================================================================

def _chunks(NLv):
    out = []
    for c0 in range(0, NLv, CHL):
        out.append(list(range(c0, min(c0 + CHL, NLv))))
    return out


def _build_program(P=128, S=8, NLv=NL, reps=1, nstr=NSTR, mode="full"):
    """Build the Bass/Tile program. Parameterized so a small variant can be
    simulated; production uses (128, 8, 63, 1, 2). reps>1 wraps the whole
    computation in an on-device For_i loop for wall-clock device timing.
    mode: 'full' | 'noscat' (scatter->memset) | 'onedve' (skip per-level stt)
    — timing-bisect variants (wrong results)."""
    import contextlib

    import concourse.bass as bass
    import concourse.mybir as mybir
    from concourse.bacc import Bacc
    from concourse.tile import TileContext

    f32 = mybir.dt.float32
    i16 = mybir.dt.int16
    i8 = mybir.dt.int8
    u16 = mybir.dt.uint16
    Alu = mybir.AluOpType
    Ax = mybir.AxisListType
    AFT = mybir.ActivationFunctionType

    SSTR = S // nstr                     # graph slots per stream
    EW = NLv * S * DEG                   # edge slots per partition
    LW = S * DEG                         # edge slots per level (all streams)
    SW = SSTR * DEG                      # edge slots per level per stream
    TRI = (NLv * (NLv + 1)) // 2
    VWS = (NLv + 1) * SSTR               # value-table width per stream (f32)
    chunks = _chunks(NLv)

    def w8l(l):                          # idx8 width of level l (one stream)
        return (l + 1) * SSTR

    def off8(l):                         # idx8 offset of level l within stream
        return (l * (l + 1)) // 2 * SSTR

    nc = Bacc()
    d_feats = nc.declare_dram_parameter("feats_p", [P, 4 * EW], f32, isOutput=False)
    d_idx = nc.declare_dram_parameter("idx8", [P, nstr * TRI * SSTR], i8, isOutput=False)
    d_cont = nc.declare_dram_parameter("cont", [P, EW], i8, isOutput=False)
    d_wb = nc.declare_dram_parameter("wb", [P, 8], f32, isOutput=False)
    d_vo = nc.declare_dram_parameter("value_o", [P, nstr * VWS], f32, isOutput=True)
    d_uo = nc.declare_dram_parameter("util_o", [P, EW], f32, isOutput=True)

    with TileContext(nc) as tc:
        with (
            tc.tile_pool(name="main", bufs=1) as pool,
            tc.tile_pool(name="lvl", bufs=4) as lp,
            tc.tile_pool(name="ichunk", bufs=3) as icp,
        ):
            feats_t = pool.tile([P, 4 * EW], f32)
            idx8_t = pool.tile([P, nstr * TRI * SSTR], i8)
            cont_t = pool.tile([P, EW], i8)
            wb_t = pool.tile([P, 8], f32)
            Uraw = pool.tile([P, EW], f32)
            Uwk = pool.tile([P, EW], f32)
            Vs = [pool.tile([P, VWS], f32, name=f"V{t}") for t in range(nstr)]

            nc.sync.dma_start(out=wb_t[:], in_=d_wb[:])
            nc.sync.dma_start(out=idx8_t[:], in_=d_idx[:])

            def w(c):
                return wb_t[:, c : c + 1]

            loop_ctx = tc.For_i(0, reps, 1) if reps > 1 else contextlib.nullcontext()
            if mode == "empty":
                with loop_ctx:
                    nc.gpsimd.memset(Vs[0][:, 0:8], 0.0)
                for t in range(nstr):
                    nc.sync.dma_start(
                        out=d_vo[:, t * VWS : (t + 1) * VWS], in_=Vs[t][:]
                    )
                nc.sync.dma_start(out=d_uo[:, 0:EW], in_=Uwk[:])
                nc.finalize()
                return nc
            with loop_ctx:
                for t in range(nstr):
                    nc.gpsimd.memset(Vs[t][:, NLv * SSTR : VWS], 0.0)
                V16 = [Vs[t][:].bitcast(u16) for t in range(nstr)]

                for ci, chunk in enumerate(chunks):
                    l0, l1 = chunk[0], chunk[-1] + 1
                    e0, e1 = l0 * LW, l1 * LW          # edge-slot range
                    # --- stream in this chunk's features / masks ---------
                    fap_s = d_feats[:].rearrange("p (c e) -> p c e", c=4)[:, :, e0:e1]
                    fap_d = feats_t[:].rearrange("p (c e) -> p c e", c=4)[:, :, e0:e1]
                    nc.sync.dma_start(out=fap_d, in_=fap_s)
                    nc.sync.dma_start(out=cont_t[:, e0:e1], in_=d_cont[:, e0:e1])

                    # --- util for this chunk on DVE ----------------------
                    def fch(c):
                        return feats_t[:, c * EW + e0 : c * EW + e1]

                    nc.vector.tensor_scalar(
                        out=Uraw[:, e0:e1], in0=fch(0),
                        scalar1=w(0), scalar2=w(4), op0=Alu.mult, op1=Alu.add,
                    )
                    for c in range(1, 4):
                        nc.vector.scalar_tensor_tensor(
                            out=Uraw[:, e0:e1], in0=fch(c),
                            scalar=w(c), in1=Uraw[:, e0:e1],
                            op0=Alu.mult, op1=Alu.add,
                        )
                    nc.sync.dma_start(out=d_uo[:, e0:e1], in_=Uraw[:, e0:e1])

                    # shift on ACT, segmented-max scan, duplicate penalty
                    nc.scalar.activation(
                        out=Uwk[:, e0:e1], in_=Uraw[:, e0:e1],
                        func=AFT.Identity, bias=w(5), scale=1.0,
                    )
                    nc.vector.tensor_tensor_scan(
                        out=Uwk[:, e0:e1], data0=cont_t[:, e0:e1],
                        data1=Uwk[:, e0:e1],
                        initial=0.0, op0=Alu.mult, op1=Alu.max,
                    )
                    nc.vector.scalar_tensor_tensor(
                        out=Uwk[:, e0 : e1 - 1], in0=cont_t[:, e0 + 1 : e1],
                        scalar=float(PEN), in1=Uwk[:, e0 : e1 - 1],
                        op0=Alu.mult, op1=Alu.add,
                    )

                    # --- expand int8 indices to interleaved int16 on ACT -
                    w8c = off8(l1) - off8(l0)          # chunk idx8 width
                    itiles = []
                    for t in range(nstr):
                        it = icp.tile([P, 2 * w8c], i16, tag=f"ic{t}")
                        src8 = idx8_t[:, t * TRI * SSTR + off8(l0) :
                                      t * TRI * SSTR + off8(l1)]
                        ev = it[:].rearrange("p (n two) -> p n two", two=2)
                        nc.scalar.activation(
                            out=ev[:, :, 0:1], in_=src8,
                            func=AFT.Copy, scale=2.0,
                        )
                        nc.scalar.activation(
                            out=ev[:, :, 1:2], in_=src8,
                            func=AFT.Identity, bias=1.0, scale=2.0,
                        )
                        # Pool observes the ACT build via a tiny copy so the
                        # scatters below carry a single (DVE) wait — the ISA
                        # encoding has one sync-wait slot.
                        ptok = lp.tile([P, 2], i16, tag=f"ptok{t}")
                        nc.gpsimd.tensor_copy(out=ptok[:], in_=it[:, 0:2])
                        itiles.append(it)

                    # --- the sweep: per level, per stream ----------------
                    for l in chunk:
                        i = NLv - 1 - l
                        loc = 2 * (off8(l) - off8(l0))
                        for t in range(nstr):
                            msg = lp.tile([P, SW], f32, tag=f"msg{t}")
                            msg16 = msg[:].bitcast(u16)
                            if mode == "noscat":
                                nc.gpsimd.memset(msg16, 0)
                            else:
                                nc.gpsimd.local_scatter(
                                    out_ap=msg16,
                                    data_ap=V16[t][:, (i + 1) * 2 * SSTR : 2 * VWS],
                                    idxs_ap=itiles[t][:, loc : loc + 2 * w8l(l)],
                                    channels=P,
                                    num_elems=2 * SW,
                                    num_idxs=2 * w8l(l),
                                )
                            u0 = l * LW + t * SW
                            if mode == "onedve":
                                m2 = msg
                            else:
                                m2 = lp.tile([P, SW], f32, tag=f"m2{t}")
                                nc.vector.scalar_tensor_tensor(
                                    out=m2[:], in0=Uwk[:, u0 : u0 + SW],
                                    scalar=w(6), in1=msg[:],
                                    op0=Alu.add, op1=Alu.add,
                                )
                            nc.vector.tensor_reduce(
                                out=Vs[t][:, i * SSTR : (i + 1) * SSTR],
                                in_=m2[:].rearrange("p (s k) -> p s k", k=DEG),
                                axis=Ax.X, op=Alu.max,
                            )

                for t in range(nstr):
                    nc.sync.dma_start(
                        out=d_vo[:, t * VWS : (t + 1) * VWS], in_=Vs[t][:]
                    )

    nc.finalize()
    return nc


def _build_program_v4(P=128, S=8, NLv=NL, reps=1):
    """v4: per-level local_scatter writes edge UTILITIES (uint16 halves of
    util+C0) into a table-shaped dst indexed by (row r=63-tgt, graph slot);
    the DVE then does masked-penalty + V-add + max-reduce over the table.
    Scatters depend only on the util prep (not on V), so the Pool engine
    runs ahead freely and the per-level recurrence lives entirely inside
    the in-order DVE stream — no per-level cross-engine round trip."""
    import contextlib

    import concourse.bass as bass
    import concourse.mybir as mybir
    from concourse.bacc import Bacc
    from concourse.tile import TileContext

    f32 = mybir.dt.float32
    i16 = mybir.dt.int16
    i8 = mybir.dt.int8
    u16 = mybir.dt.uint16
    Alu = mybir.AluOpType
    Ax = mybir.AxisListType
    AFT = mybir.ActivationFunctionType

    EW = NLv * S * DEG                   # edge slots per partition
    LW = S * DEG                         # edge slots per level
    VW = (NLv + 1) * S                   # value table (reversed rows)
    chunks = _chunks(NLv)
    TMW = sum(len(ch) * (ch[-1] + 1) * S for ch in chunks)

    nc = Bacc()
    d_feats = nc.declare_dram_parameter("feats_p", [P, 4 * EW], f32, isOutput=False)
    d_idx = nc.declare_dram_parameter("idx16", [P, NLv * 2 * LW], i16, isOutput=False)
    d_tm = nc.declare_dram_parameter("tmask", [P, TMW], i8, isOutput=False)
    d_cont = nc.declare_dram_parameter("cont", [P, EW], i8, isOutput=False)
    d_wb = nc.declare_dram_parameter("wb", [P, 8], f32, isOutput=False)
    d_vo = nc.declare_dram_parameter("value_o", [P, VW], f32, isOutput=True)
    d_uo = nc.declare_dram_parameter("util_o", [P, EW], f32, isOutput=True)

    with TileContext(nc) as tc:
        with (
            tc.tile_pool(name="main", bufs=1) as pool,
            tc.tile_pool(name="ctp", bufs=2) as lp,
        ):
            feats_t = pool.tile([P, 4 * EW], f32)
            idx_t = pool.tile([P, NLv * 2 * LW], i16)
            tm_t = pool.tile([P, TMW], i8)
            cont_t = pool.tile([P, EW], i8)
            wb_t = pool.tile([P, 8], f32)
            Uraw = pool.tile([P, EW], f32)
            Uwk = pool.tile([P, EW], f32)
            V = pool.tile([P, VW], f32)

            nc.sync.dma_start(out=wb_t[:], in_=d_wb[:])
            nc.sync.dma_start(out=idx_t[:], in_=d_idx[:])
            nc.sync.dma_start(out=tm_t[:], in_=d_tm[:])

            def w(c):
                return wb_t[:, c : c + 1]

            loop_ctx = tc.For_i(0, reps, 1) if reps > 1 else contextlib.nullcontext()
            with loop_ctx:
                nc.vector.memset(V[:, 0:S], 0.0)             # node 63 row
                Uwk16 = Uwk[:].bitcast(u16)

                tm_off = 0
                for ci, chunk in enumerate(chunks):
                    l0, l1 = chunk[0], chunk[-1] + 1
                    nl = len(chunk)
                    Wc = l1 * S                      # padded table width
                    e0, e1 = l0 * LW, l1 * LW
                    fap_s = d_feats[:].rearrange("p (c e) -> p c e", c=4)[:, :, e0:e1]
                    fap_d = feats_t[:].rearrange("p (c e) -> p c e", c=4)[:, :, e0:e1]
                    nc.sync.dma_start(out=fap_d, in_=fap_s)
                    nc.sync.dma_start(out=cont_t[:, e0:e1], in_=d_cont[:, e0:e1])

                    def fch(c):
                        return feats_t[:, c * EW + e0 : c * EW + e1]

                    nc.vector.tensor_scalar(
                        out=Uraw[:, e0:e1], in0=fch(0),
                        scalar1=w(0), scalar2=w(4), op0=Alu.mult, op1=Alu.add,
                    )
                    for c in range(1, 4):
                        nc.vector.scalar_tensor_tensor(
                            out=Uraw[:, e0:e1], in0=fch(c),
                            scalar=w(c), in1=Uraw[:, e0:e1],
                            op0=Alu.mult, op1=Alu.add,
                        )
                    nc.sync.dma_start(out=d_uo[:, e0:e1], in_=Uraw[:, e0:e1])
                    nc.scalar.activation(
                        out=Uwk[:, e0:e1], in_=Uraw[:, e0:e1],
                        func=AFT.Identity, bias=w(5), scale=1.0,
                    )
                    nc.vector.tensor_tensor_scan(
                        out=Uwk[:, e0:e1], data0=cont_t[:, e0:e1],
                        data1=Uwk[:, e0:e1],
                        initial=0.0, op0=Alu.mult, op1=Alu.max,
                    )
                    nc.vector.scalar_tensor_tensor(
                        out=Uwk[:, e0 : e1 - 1], in0=cont_t[:, e0 + 1 : e1],
                        scalar=float(PEN), in1=Uwk[:, e0 : e1 - 1],
                        op0=Alu.mult, op1=Alu.add,
                    )

                    # padded chunk table; each scatter zeroes its whole
                    # padded row, so pads are PEN-masked zeros
                    CT = lp.tile([P, nl * Wc], f32, tag="ct")
                    for li, l in enumerate(chunk):
                        nc.gpsimd.local_scatter(
                            out_ap=CT[:, li * Wc : (li + 1) * Wc].bitcast(u16),
                            data_ap=Uwk16[:, l * LW * 2 : (l + 1) * LW * 2],
                            idxs_ap=idx_t[:, l * 2 * LW : (l + 1) * 2 * LW],
                            channels=P,
                            num_elems=2 * Wc,
                            num_idxs=2 * LW,
                        )
                    # chunk-wide empty-slot penalty
                    nc.vector.scalar_tensor_tensor(
                        out=CT[:], in0=tm_t[:, tm_off : tm_off + nl * Wc],
                        scalar=float(PEN), in1=CT[:],
                        op0=Alu.mult, op1=Alu.add,
                    )
                    # chunk-wide V-add for rows known before this chunk
                    if l0 > 0:
                        ct3 = CT[:].rearrange("p (t w) -> p t w", w=Wc)
                        vb = V[:, 0 : l0 * S].rearrange(
                            "p (o w) -> p o w", o=1
                        ).broadcast_to([P, nl, l0 * S])
                        nc.vector.scalar_tensor_tensor(
                            out=ct3[:, :, 0 : l0 * S],
                            in0=ct3[:, :, 0 : l0 * S],
                            scalar=w(6),
                            in1=vb,
                            op0=Alu.add, op1=Alu.add,
                        )
                    for li, l in enumerate(chunk):
                        wl = l + 1
                        lo = li * Wc + l0 * S
                        nc.vector.scalar_tensor_tensor(
                            out=CT[:, lo : li * Wc + wl * S],
                            in0=CT[:, lo : li * Wc + wl * S],
                            scalar=w(6), in1=V[:, l0 * S : wl * S],
                            op0=Alu.add, op1=Alu.add,
                        )
                        nc.vector.tensor_reduce(
                            out=V[:, wl * S : (wl + 1) * S],
                            in_=CT[:, li * Wc : li * Wc + wl * S].rearrange(
                                "p (r s) -> p s r", s=S),
                            axis=Ax.X, op=Alu.max,
                        )
                    tm_off += nl * Wc

                nc.sync.dma_start(out=d_vo[:], in_=V[:])

    nc.finalize()
    return nc


def _get_program(key=(128, 8, NL, 1)):
    if key not in _PROG_CACHE:
        if len(key) > 4 and key[4] == "v4":
            _PROG_CACHE[key] = _build_program_v4(*key[:4])
        else:
            _PROG_CACHE[key] = _build_program(*key)
    return _PROG_CACHE[key]


# =====================================================================
# entry point
# =====================================================================

def _host_v4(feats, edge_index, W, b):
    tgt_l = (edge_index[1] - edge_index[0] // N * N).astype(np.int32)
    feats_p, idx16, tmask, cont_p, wb, perm = _prepare_host_v4(
        feats, tgt_l.reshape(NG, NL, DEG), W, b, NCORES, 128, 8, NL, 1
    )
    in_maps = [
        {
            "feats_p": feats_p[c],
            "idx16": idx16[c],
            "tmask": tmask[c],
            "cont": cont_p[c],
            "wb": wb[c],
        }
        for c in range(NCORES)
    ]
    return in_maps, perm


def _run_device(feats, edge_index, W, b, trace=False):
    from concourse.bass_utils import run_bass_kernel_spmd

    in_maps, perm = _host_v4(feats, edge_index, W, b)
    nc = _get_program((128, 8, NL, 1, "v4"))
    res = run_bass_kernel_spmd(nc, in_maps, list(range(NCORES)), trace=trace)
    return res, perm


def _pjrt_loop_time(nc, in_maps, iters):
    """Build the sharded PJRT executable for `nc`, pre-stage inputs on the
    devices, and return the best wall time (s) of one execution."""
    import time as _time

    import jax
    import numpy as _np
    from jax.sharding import Mesh, NamedSharding, PartitionSpec

    try:
        from jax.experimental.shard_map import shard_map
    except Exception:
        from jax.shard_map import shard_map  # newer jax

    import concourse.mybir as mybir
    from concourse import bass2jax as b2j

    b2j.install_neuronx_cc_hook()
    partition_name = nc.partition_id_tensor.name if nc.partition_id_tensor else None
    in_names, out_names, out_avals, zero_outs = [], [], [], []
    for alloc in nc.m.functions[0].allocations:
        if not isinstance(alloc, mybir.MemoryLocationSet):
            continue
        name = alloc.memorylocations[0].name
        if alloc.kind == "ExternalInput":
            if name != partition_name:
                in_names.append(name)
        elif alloc.kind == "ExternalOutput":
            out_names.append(name)
            out_avals.append(
                jax.core.ShapedArray(tuple(alloc.tensor_shape), mybir.dt.np(alloc.dtype))
            )
            zero_outs.append(
                _np.zeros(tuple(alloc.tensor_shape), mybir.dt.np(alloc.dtype))
            )
    n_params = len(in_names)
    all_names = list(in_names) + list(out_names)
    if partition_name is not None:
        all_names.append(partition_name)

    def _body(*args):
        operands = list(args)
        if partition_name is not None:
            operands.append(b2j.partition_id_tensor())
        return tuple(
            b2j._bass_exec_p.bind(
                *operands,
                out_avals=tuple(out_avals),
                in_names=tuple(all_names),
                out_names=tuple(out_names),
                lowering_input_output_aliases=(),
                sim_require_finite=True,
                sim_require_nnan=True,
                nc=nc,
            )
        )

    n_outs = len(out_names)
    donate = tuple(range(n_params, n_params + n_outs))
    devices = jax.devices()[:NCORES]
    mesh = Mesh(_np.asarray(devices), ("core",))
    spec = PartitionSpec("core")
    sharded = jax.jit(
        shard_map(
            _body, mesh=mesh,
            in_specs=(spec,) * (n_params + n_outs),
            out_specs=(spec,) * n_outs,
            check_rep=False,
        ),
        donate_argnums=donate,
        keep_unused=True,
    )
    sh = NamedSharding(mesh, spec)
    concat_in = [
        jax.device_put(
            _np.concatenate([in_maps[c][nm] for c in range(NCORES)], axis=0), sh
        )
        for nm in in_names
    ]
    concat_zeros = [
        _np.zeros((NCORES * z.shape[0], *z.shape[1:]), z.dtype) for z in zero_outs
    ]
    # warm (compile + first exec)
    jax.block_until_ready(sharded(*concat_in, *concat_zeros))
    best = float("inf")
    for _ in range(iters):
        zs = [jax.device_put(z, sh) for z in concat_zeros]
        jax.block_until_ready(zs)
        t0 = _time.perf_counter()
        jax.block_until_ready(sharded(*concat_in, *zs))
        best = min(best, _time.perf_counter() - t0)
    return best


def timed_run(np_inputs, lo=8, hi=264, iters=6, verbose=True, mode="full"):
    """Estimate single-iteration device time by wall-clocking two on-device
    looped variants (For_i with `lo` and `hi` trip counts) and taking the
    delta — per-call dispatch overhead cancels. Returns ns."""
    in_maps, _ = _host_v4(
        np.asarray(np_inputs["feats"]),
        np.asarray(np_inputs["edge_index"]),
        np.asarray(np_inputs["W"]),
        np.asarray(np_inputs["b"]),
    )
    walls = {}
    for reps in (lo, hi):
        nc = _get_program((128, 8, NL, reps, "v4"))
        walls[reps] = _pjrt_loop_time(nc, in_maps, iters)
        if verbose:
            print(f"  loop reps={reps}: best wall {walls[reps] * 1e3:.2f} ms")
    return int((walls[hi] - walls[lo]) / (hi - lo) * 1e9)


def kernel(feats, dest_mask, edge_index, W, b, n_steps):
    feats = np.asarray(feats)
    edge_index = np.asarray(edge_index)
    W = np.asarray(W)
    b = np.asarray(b)
    if not _check_structure(feats, dest_mask, edge_index, W, b, n_steps):
        return _reference_fallback(feats, dest_mask, edge_index, W, b, n_steps)

    res, perm = _run_device(feats, edge_index, W, b)
    results = res.results

    # ---- assemble outputs ----------------------------------------------
    value = np.empty((NG, N), np.float32)
    util = np.empty((E,), np.float32)
    for c in range(NCORES):
        vo = results[c]["value_o"]                 # [128, 64*8] reversed rows
        vr = vo.reshape(128, N, 8)[:, ::-1, :].transpose(0, 2, 1)
        value[c * GPC : (c + 1) * GPC] = vr.reshape(GPC, N)
        util[perm[c].reshape(-1)] = results[c]["util_o"].reshape(-1)
    return value.reshape(NG * N, 1), util.reshape(E, 1)
